# revision 21
# baseline (speedup 1.0000x reference)
"""Trainium2 Bass kernel for a single-layer LSTM.

Problem: x [64, 1024, 512] f32, W [512, 2048], U [512, 2048], bias [2048].
  xW = einsum('bsi,ig->sbg', x, W) + bias
  per step: gates = xW[t] + h @ U ; i,f,o = sigmoid ; g = tanh
            c = f*c + i*g ; h = o*tanh(c)
Returns (hidden_seq [64,1024,512], h_t [64,512], c_t [64,512]).

Sharding: data-parallel over batch across 8 cores (8 samples/core),
W/U/bias replicated. Sequence stays on-core (recurrence).

Per-core kernel design:
  - All matmuls run as float32r (full-rate streaming on the PE at N=512).
  - Recurrent matmul keeps batch (8) on the PSUM partition axis and streams
    U as the moving operand; the 4 gate chunks (i,f,o,g) are computed by
    4 concurrent column-tiled matmul groups (tile_position=(0,32j)) landing
    on partition strips {0,32,64,96}+[0,8) of ONE PSUM bank, so one
    512-column stream time covers all four chunks.
  - xW[t]+bias is injected into the same accumulation group via an
    identity-matmul (K=8) from an SBUF tile, so no separate VE add.
  - sigmoid(i,f,o) is ONE activation op spanning partitions 0..71.
  - c is updated batch-major [8,512] on VE; c and o are then PE-transposed
    to [128, 32] (hidden-major) so tanh(c) and h = o*tanh(c) run on the
    cheap layout, and h_T directly provides the next step's stationary
    operand (no extra transpose on the critical path).
  - h_T accumulates 16 steps in SBUF, then one 256KB DMA per block writes
    the hidden sequence to DRAM (decoded to [B,S,H] on the host).
"""

import numpy as np

import concourse.bass as bass
import concourse.mybir as mybir
import concourse.tile as tile
from concourse import bacc
from concourse.bass_utils import run_bass_kernel_spmd

F32 = mybir.dt.float32
F32R = mybir.dt.float32r

N_CORES = 8
B_FULL = 64
BC = B_FULL // N_CORES      # batch per core = 8
S_FULL = 1024
I_SZ = 512                  # input size
H_SZ = 512                  # hidden size
G_SZ = 4 * H_SZ             # gates = 2048
KK = I_SZ // 128            # k-chunks = 4
NJ = G_SZ // 512            # gate chunks = 4 (i, f, o, g strips)
HBLOCK = 16                 # steps of h_T per output DMA block

SIG = mybir.ActivationFunctionType.Sigmoid
TANH = mybir.ActivationFunctionType.Tanh

# gate order inside the PSUM bank partition strips:
#   strip 0 (part   0- 7): i
#   strip 1 (part  32-39): f
#   strip 2 (part  64-71): o
#   strip 3 (part  96-103): g
# W/U columns are reordered on the host so that gate chunk j of the
# on-device layout maps to U[:, perm] accordingly: chunks (i, f, o, g).
# Reference layout is (i, f, g, o) -> host passes permuted W/U/bias.


def _r(ap):
    return ap.bitcast(F32R)


def _build(n_steps: int, col_tile: bool = True):
    """Build the per-core Bass module. Returns the Bacc object."""
    nc = bacc.Bacc(
        "TRN2",
        target_bir_lowering=False,
        debug=False,
        enable_asserts=False,
    )

    s_steps = n_steps
    n_blocks = (s_steps + HBLOCK - 1) // HBLOCK

    x_d = nc.dram_tensor("x", [BC, s_steps, I_SZ], F32, kind="ExternalInput").ap()
    w_d = nc.dram_tensor("W", [I_SZ, G_SZ], F32, kind="ExternalInput").ap()
    u_d = nc.dram_tensor("U", [I_SZ, G_SZ], F32, kind="ExternalInput").ap()
    b_d = nc.dram_tensor("bias", [1, G_SZ], F32, kind="ExternalInput").ap()
    id_d = nc.dram_tensor("ident", [129, 128], F32, kind="ExternalInput").ap()
    e8_d = nc.dram_tensor("eye8s", [128, BC], F32, kind="ExternalInput").ap()

    xw_d = nc.dram_tensor("xw_scratch", [s_steps, BC, G_SZ], F32, kind="Internal").ap()
    hid_d = nc.dram_tensor(
        "hid", [n_blocks, 128, HBLOCK, KK, BC], F32, kind="ExternalOutput"
    ).ap()
    c_out_d = nc.dram_tensor(
        "c_out", [128, KK * BC], F32, kind="ExternalOutput"
    ).ap()

    with tile.TileContext(nc) as tc:
        with tc.tile_pool(name="consts", bufs=1) as cpool:
            u_sb = cpool.tile([128, KK, G_SZ], F32R)      # U[kk*128+p, n]
            w_sb = cpool.tile([128, KK, G_SZ], F32R)
            bias_sb = cpool.tile([1, G_SZ], F32R)
            ident_sb = cpool.tile([128, 128], F32)
            eye8s_sb = cpool.tile([128, BC], F32)
            eyer_sb = cpool.tile([BC, BC], F32R)
            ones_sb = cpool.tile([1, 128], F32R)
            c_sb = cpool.tile([128, KK * BC], F32)       # cell state, hidden-major

            for kk in range(KK):
                nc.sync.dma_start(u_sb[:, kk, :], u_d[kk * 128:(kk + 1) * 128, :].bitcast(F32R))
                nc.sync.dma_start(w_sb[:, kk, :], w_d[kk * 128:(kk + 1) * 128, :].bitcast(F32R))
            nc.sync.dma_start(bias_sb[:], b_d[:].bitcast(F32R))
            nc.sync.dma_start(ident_sb[:], id_d[0:128, :])
            nc.sync.dma_start(eye8s_sb[:], e8_d[:])
            nc.sync.dma_start(eyer_sb[:], id_d[0:BC, 0:BC].bitcast(F32R))
            nc.sync.dma_start(ones_sb[:], id_d[128:129, :].bitcast(F32R))
            nc.gpsimd.memset(c_sb[:], 0.0)

            # ---------------- Phase B: xW = x @ W + bias -> DRAM -------------
            mtiles = s_steps // 128 if s_steps >= 128 else 0
            rem = s_steps - mtiles * 128
            with (
                tc.tile_pool(name="xin", bufs=3) as xin_pool,
                tc.tile_pool(name="xtp", bufs=2, space="PSUM") as xtp_pool,
                tc.tile_pool(name="xT", bufs=2) as xT_pool,
                tc.tile_pool(name="xwps", bufs=6, space="PSUM") as xwps_pool,
                tc.tile_pool(name="xwsb", bufs=3) as xwsb_pool,
            ):
                def xw_tile(b, s0, sn):
                    """Compute xw rows [s0, s0+sn) for batch b."""
                    xt = xin_pool.tile([128, I_SZ], F32, tag="xt")
                    nc.sync.dma_start(xt[:sn, :], x_d[b, s0:s0 + sn, :])
                    xT = xT_pool.tile([128, KK, 128], F32R, tag="xT")
                    for kk in range(KK):
                        xtp = xtp_pool.tile([128, 128], F32, tag="xtp")
                        nc.tensor.transpose(
                            xtp[:, :sn], xt[:sn, kk * 128:(kk + 1) * 128],
                            ident_sb[:sn, :sn],
                        )
                        nc.vector.tensor_copy(xT[:, kk, :sn], xtp[:, :sn])
                    xw_sb = xwsb_pool.tile([128, G_SZ], F32R, tag="xwsb")
                    for n in range(NJ):
                        ps = xwps_pool.tile([128, 512], F32, tag="xwps")
                        nc.tensor.matmul(
                            ps[:sn, :], ones_sb[:, :sn],
                            bias_sb[:, n * 512:(n + 1) * 512],
                            start=True, stop=False,
                        )
                        for kk in range(KK):
                            nc.tensor.matmul(
                                ps[:sn, :], xT[:, kk, :sn],
                                u_rhs(w_sb, kk, n),
                                start=False, stop=(kk == KK - 1),
                            )
                        nc.scalar.activation(
                            xw_sb[:sn, n * 512:(n + 1) * 512], ps[:sn, :],
                            mybir.ActivationFunctionType.Identity,
                        )
                    nc.sync.dma_start(xw_d[s0:s0 + sn, b, :].bitcast(F32R), xw_sb[:sn, :])

                def u_rhs(t, kk, n):
                    return t[:, kk, n * 512:(n + 1) * 512]

                for b in range(BC):
                    for mt in range(mtiles):
                        xw_tile(b, mt * 128, 128)
                    if rem:
                        xw_tile(b, mtiles * 128, rem)

            # ---------------- Phase C: recurrence ---------------------------
            # Gate strips in one PSUM bank: i@0, f@32, o@64, g@96 (col-tiled).
            # All elementwise tail work runs in the transposed (hidden-major)
            # [128, KK, BC] domain at base partition 0: the four gate strips
            # are PE-transposed into one PSUM bank laid out
            # [128, kk, gate, b], and c lives transposed in SBUF.
            with (
                tc.tile_pool(name="xwb", bufs=4) as xwb_pool,
                tc.tile_pool(name="gps", bufs=1, space="PSUM") as gps_pool,
                tc.tile_pool(name="tps", bufs=2, space="PSUM") as tps_pool,
                tc.tile_pool(name="sig", bufs=2) as sig_pool,
                tc.tile_pool(name="tg", bufs=2) as tg_pool,
                tc.tile_pool(name="tmp", bufs=4) as tmp_pool,
                tc.tile_pool(name="tct", bufs=2) as tct_pool,
                tc.tile_pool(name="hout", bufs=2) as hout_pool,
            ):
                hT_read = None      # SBUF AP [128, KK, BC] with step t-1's h_T
                hout_cur = None
                cT = c_sb[:].rearrange("p (kk b) -> p kk b", kk=KK)

                for t in range(s_steps):
                    slot = t % HBLOCK
                    if slot == 0:
                        hout_cur = hout_pool.tile(
                            [128, HBLOCK, KK, BC], F32R, tag="hout"
                        )

                    # xw(t) (+bias already folded) -> SBUF
                    xwb = xwb_pool.tile([BC, G_SZ], F32R, tag="xwb")
                    nc.sync.dma_start(xwb[:], xw_d[t, :, :].bitcast(F32R))

                    # gates: 4 chunks (i,f,o,g) in 4 PSUM banks, base
                    # partition 0
                    gates = gps_pool.tile([BC, NJ, 512], F32, tag="gates")
                    for j in range(NJ):
                        nc.tensor.matmul(
                            gates[:, j, :],
                            eyer_sb[:],
                            xwb[:, j * 512:(j + 1) * 512],
                            start=True, stop=(t == 0),
                            skip_group_check=True,
                        )
                    if t > 0:
                        for kk in range(KK):
                            for j in range(NJ):
                                nc.tensor.matmul(
                                    gates[:, j, :],
                                    hT_read[:, kk, :],
                                    u_rhs(u_sb, kk, j),
                                    start=False, stop=(kk == KK - 1),
                                    skip_group_check=True,
                                )

                    # activations: tanh for g first (t1 needs it), then one
                    # sigmoid covering the i,f,o banks
                    tg = tg_pool.tile([BC, 512], F32, tag="tg")
                    nc.scalar.activation(tg[:], gates[:, 3, :], TANH)
                    sg = sig_pool.tile([BC, 3, 512], F32, tag="sig")
                    for j in range(3):
                        nc.scalar.activation(sg[:, j, :], gates[:, j, :], SIG)

                    # transposed-gate bank layout: [128, kk, (i,g,f,o), b]
                    tps = tps_pool.tile([128, KK, 4, BC], F32, tag="tps")
                    e8 = ident_sb[:BC, :BC]
                    for kk in range(KK):
                        ck = slice(kk * 128, (kk + 1) * 128)
                        nc.tensor.transpose(tps[:, kk, 1, :], tg[:, ck], e8)
                        nc.tensor.transpose(tps[:, kk, 0, :], sg[:, 0, ck], e8)
                    for kk in range(KK):
                        ck = slice(kk * 128, (kk + 1) * 128)
                        nc.tensor.transpose(tps[:, kk, 2, :], sg[:, 1, ck], e8)
                        nc.tensor.transpose(tps[:, kk, 3, :], sg[:, 2, ck], e8)
                    fT = tps[:, :, 2, :]
                    oT = tps[:, :, 3, :]

                    # i,g pair -> SBUF (DVE can read at most one PSUM operand)
                    ig_sb = tmp_pool.tile([128, KK, 2, BC], F32, tag="ig")
                    nc.scalar.activation(
                        ig_sb[:], tps[:, :, 0:2, :],
                        mybir.ActivationFunctionType.Identity,
                    )

                    # c_T = f_T*c_T + i_T*g_T  (hidden-major, free size 32)
                    t1 = tmp_pool.tile([128, KK, BC], F32, tag="t1")
                    nc.vector.tensor_mul(t1[:], ig_sb[:, :, 0, :], ig_sb[:, :, 1, :])
                    if t == 0:
                        nc.vector.tensor_copy(cT, t1[:])
                    else:
                        t2 = tmp_pool.tile([128, KK, BC], F32, tag="t2")
                        nc.vector.tensor_mul(t2[:], fT, cT)
                        nc.vector.tensor_add(cT, t1[:], t2[:])

                    tct = tct_pool.tile([128, KK, BC], F32, tag="tct")
                    nc.scalar.activation(tct[:], cT, TANH)

                    # h_T = o_T * tanh(c_T) -> hout slot (also next lhsT)
                    hslot = hout_cur[:, slot, :, :]
                    nc.vector.tensor_mul(hslot, oT, tct[:])
                    hT_read = hslot

                    if slot == HBLOCK - 1 or t == s_steps - 1:
                        nc.sync.dma_start(
                            hid_d[t // HBLOCK].bitcast(F32R), hout_cur[:, :, :, :]
                        )

                nc.sync.dma_start(c_out_d[:], c_sb[:])

    nc.compile()
    return nc


_BUILD_CACHE = {}


def _get_nc(n_steps: int, col_tile: bool = True):
    key = (n_steps, col_tile)
    if key not in _BUILD_CACHE:
        _BUILD_CACHE[key] = _build(n_steps, col_tile)
    return _BUILD_CACHE[key]


# host-side gate permutation: reference gate order is (i, f, g, o) along the
# 2048 axis; the device layout wants (i, f, o, g).
_PERM = np.concatenate([
    np.arange(0, 512),            # i
    np.arange(512, 1024),         # f
    np.arange(1536, 2048),        # o
    np.arange(1024, 1536),        # g
])


def _decode_hidden(hid: np.ndarray, n_steps: int) -> np.ndarray:
    """[n_blocks, 128, HBLOCK*KK*BC] -> [BC, n_steps, H]."""
    n_blocks = hid.shape[0]
    a = hid.reshape(n_blocks, 128, HBLOCK, KK, BC)
    a = a.transpose(4, 0, 2, 3, 1)           # [b, blk, s16, kk, p]
    a = a.reshape(BC, n_blocks * HBLOCK, H_SZ)
    return a[:, :n_steps]


def kernel(x, W, U, bias, n_steps: int = S_FULL, col_tile: bool = True,
           trace: bool = False):
    x = np.asarray(x, dtype=np.float32)
    W = np.asarray(W, dtype=np.float32)
    U = np.asarray(U, dtype=np.float32)
    bias = np.asarray(bias, dtype=np.float32)

    nc = _get_nc(n_steps, col_tile)

    Wp = np.ascontiguousarray(W[:, _PERM])
    Up = np.ascontiguousarray(U[:, _PERM])
    bp = np.ascontiguousarray(bias[_PERM].reshape(1, -1))
    ident = np.zeros((129, 128), dtype=np.float32)
    ident[:128] = np.eye(128, dtype=np.float32)
    ident[128] = 1.0
    eye8s = np.zeros((128, BC), dtype=np.float32)
    for j in range(4):
        eye8s[32 * j:32 * j + BC, :] = np.eye(BC, dtype=np.float32)

    in_maps = []
    for c in range(N_CORES):
        in_maps.append({
            "x": np.ascontiguousarray(x[c * BC:(c + 1) * BC, :n_steps]),
            "W": Wp,
            "U": Up,
            "bias": bp,
            "ident": ident,
            "eye8s": eye8s,
        })

    res = run_bass_kernel_spmd(
        nc, in_maps, core_ids=list(range(N_CORES)), trace=trace
    )
    kernel._last_results = res

    hidden = np.concatenate(
        [_decode_hidden(r["hid"], n_steps) for r in res.results], axis=0
    )
    c_t = np.concatenate(
        [r["c_out"].reshape(128, KK, BC).transpose(2, 1, 0).reshape(BC, H_SZ)
         for r in res.results], axis=0
    )
    h_t = np.ascontiguousarray(hidden[:, -1, :])
    return hidden, h_t, c_t
